# revision 34
# baseline (speedup 1.0000x reference)
"""MultiHeadAttention TRN2 Bass kernel, sharded over 8 NeuronCores.

Sharding: 8 cores = 2 batches x 4 head-groups. Each core computes 4 heads of
one batch end-to-end (q/k/v projections, biased+masked softmax attention, and
a partial output projection); the host sums the per-group partial outputs.

v3 design (single interleaved stream, ScalarE exp is the critical path):
  - all-bf16 matmuls (fp8 measured too lossy: attention does not average away
    per-element weight noise, so fp8's ~3.6% RMS passes straight to the
    output). K=128 everywhere - scores use per-head zero-padded kT tiles so
    the PE never switches tiling mode (mode switches drain the array and
    serialize weight loads).
  - exp on ScalarE with scale=1/8 (the 1/sqrt(dh)) and bias=-6*ln2 (harmless
    constant shift; cancels in the softmax normalize). 128 activations of
    [128,1024] = the ~145us critical path; ScalarE does nothing else.
  - a = e * expb (mask*exp(chem_bias), bf16) on DVE in 2x mode; attn@v with
    a ones-column denominator (even heads: ones col 64; odd heads: ones col
    0, v dims in 64:128 so every epilogue op stays partition-aligned).
  - out2 is evacuated PSUM->SBUF so the single PSUM accumulator slot frees
    for the next head; the normalize epilogue (den row -> DRAM -> [128,8]
    spread -> reciprocal -> broadcast -> scale + v-bias) is software-
    pipelined into the next head's chunk stream.
  - x tiles stream through a shared 4-slot pool; the projection units that
    consume late-arriving halves (k/v second half, q superblock 1) are
    injected into the first attention head's chunk stream, and the output
    projection (bf16 partials, summed on host) into the second superblock's.
"""

import numpy as np
import ml_dtypes

import concourse.bass as bass
import concourse.mybir as mybir
import concourse.tile as tile
from concourse.bacc import Bacc

BF16 = mybir.dt.bfloat16
F32 = mybir.dt.float32
nbf16 = ml_dtypes.bfloat16

B = 2
S = 2048
D = 1024
H = 16
DH = 64
HPC = 4  # heads per core
CD = HPC * DH  # 256 per-core projected dims
NCORES = 8

KC = D // 128  # 8 contraction chunks for projections
TC = S // 128  # 16 token (s_k) chunks
SUPS = 2
SUPLEN = S // SUPS  # 1024 columns per s_q superblock
NB = 512  # projection/outproj token block

EXP_SCALE = 0.125  # 1/sqrt(dh)
EXP_BIAS = -4.158883083359672  # -6*ln2, cancels in the normalize


def build_module(debug=False):
    nc = Bacc(None)

    xq_d = nc.dram_tensor("xq", [128, KC, S], BF16, kind="ExternalInput")
    xk_d = nc.dram_tensor("xk", [128, KC, S], BF16, kind="ExternalInput")
    xv_d = nc.dram_tensor("xv", [128, KC, S], BF16, kind="ExternalInput")
    wq_d = nc.dram_tensor("wq", [128, KC, CD], BF16, kind="ExternalInput")
    wk_d = nc.dram_tensor("wk", [128, KC, CD], BF16, kind="ExternalInput")
    wv_d = nc.dram_tensor("wv", [128, KC, CD], BF16, kind="ExternalInput")
    wo_d = nc.dram_tensor("wo", [128, CD // 128, D], BF16, kind="ExternalInput")
    bq_d = nc.dram_tensor("bq", [128, 2], F32, kind="ExternalInput")
    bk_d = nc.dram_tensor("bk", [128, 2], F32, kind="ExternalInput")
    expb_d = nc.dram_tensor("expb", [S, S], BF16, kind="ExternalInput")  # [s_k, s_q]
    pout_d = nc.dram_tensor("pout", [D, S], BF16, kind="ExternalOutput")
    if debug:
        dbg_qT = nc.dram_tensor("dbg_qT", [2, 128, S], BF16, kind="ExternalOutput")
        dbg_kT = nc.dram_tensor("dbg_kT", [HPC, 128, S], BF16, kind="ExternalOutput")
        dbg_vv = nc.dram_tensor("dbg_vv", [128, HPC * TC * 128], BF16, kind="ExternalOutput")
        dbg_cc = nc.dram_tensor("dbg_cc", [2, 128, S], BF16, kind="ExternalOutput")

    with tile.TileContext(nc) as tc:
        with (
            tc.tile_pool(name="statics", bufs=1) as statics,
            tc.tile_pool(name="xh", bufs=5) as x_pool,
            tc.tile_pool(name="expb", bufs=4) as expb_pool,
            tc.tile_pool(name="e", bufs=3) as e_pool,
            tc.tile_pool(name="a", bufs=7) as a_pool,
            tc.tile_pool(name="o2s", bufs=2) as o2s_pool,
            tc.tile_pool(name="spr", bufs=2) as spr_pool,
            tc.tile_pool(name="rb", bufs=2) as rb_pool,
            tc.tile_pool(name="oev", bufs=3) as oev_pool,
            tc.tile_pool(name="psc", bufs=2, space="PSUM") as psc,
            tc.tile_pool(name="pacc", bufs=1, space="PSUM") as pacc,
            tc.tile_pool(name="pop", bufs=2, space="PSUM") as pop,
            tc.tile_pool(name="dsc", bufs=4, space="DRAM") as dram_pool,
        ):
            # ---- statics ----
            wq_sb = statics.tile([128, KC, CD], BF16, name="wq_sb")
            wk_sb = statics.tile([128, KC, CD], BF16, name="wk_sb")
            wv_sb = statics.tile([128, KC, CD], BF16, name="wv_sb")
            wo_sb = statics.tile([128, CD // 128, D], BF16, name="wo_sb")
            bq_sb = statics.tile([128, 2], F32, name="bq_sb")
            bk_sb = statics.tile([128, 2], F32, name="bk_sb")
            bias_t = statics.tile([128, 1], F32, name="bias_t")
            qT = [statics.tile([128, S], BF16, name=f"qT{m}") for m in range(2)]
            # per-head kT, zero-padded on the other head's 64 rows so every
            # scores matmul contracts a full K=128 (no PE mode switches)
            kTh = [statics.tile([128, S], BF16, name=f"kTh{h}") for h in range(HPC)]
            cc = [statics.tile([128, S], BF16, name=f"cc{m}") for m in range(2)]
            # vv[:, vh, tk, :]: attnv lhsT per (head, s_k chunk), heads in vh
            # order (0,2,1,3; Wv host-reordered). Even-parity heads: v dims in
            # cols 0:64, ones col 64; odd parity: ones col 0, v dims 64:128.
            vv = statics.tile([128, HPC, TC, 128], BF16, name="vv")

            # critical-path memsets on DVE (the rest are emitted post-prefix)
            nc.vector.memset(bias_t, EXP_BIAS)
            nc.vector.memset(kTh[0], 0.0)
            nc.vector.memset(kTh[1], 0.0)

            # ---- input loads, spread across the three DGE-capable queues ----
            # x half-tiles stream through a 5-slot pool; alloc order:
            # [xq0, xk0, xv0, xk1, xv1] then xq1 takes xq0's freed slot.
            def x_half(src, half, eng, split=2, eng2=None):
                # eng2: alternate issuing queue per kc chunk - two queues'
                # DMA-engine shares land the tile ~2x faster
                t = x_pool.tile([128, KC, SUPLEN], BF16, name="xh")
                for q in range(split):  # first-needed column block lands first
                    w = SUPLEN // split
                    qsl_s = slice(half * SUPLEN + q * w, half * SUPLEN + (q + 1) * w)
                    qsl_d = slice(q * w, (q + 1) * w)
                    for kc in range(KC):
                        e_ = eng2 if (eng2 is not None and kc % 2 == 1) else eng
                        e_.dma_start(t[:, kc, qsl_d], src[:, kc, qsl_s])
                return t

            # scalar queue is free until the first exp: it carries xq0
            xq0 = x_half(xq_d, 0, nc.scalar)

            nc.sync.dma_start(bq_sb, bq_d[:, :])
            nc.sync.dma_start(bk_sb, bk_d[:, :])
            for j in range(4):
                nc.sync.dma_start(wq_sb[:, 2 * j : 2 * j + 2, :], wq_d[:, 2 * j : 2 * j + 2, :])
            for j in range(4):
                nc.sync.dma_start(wk_sb[:, 2 * j : 2 * j + 2, :], wk_d[:, 2 * j : 2 * j + 2, :])
            xk0 = x_half(xk_d, 0, nc.sync)
            for j in range(4):
                nc.sync.dma_start(wv_sb[:, 2 * j : 2 * j + 2, :], wv_d[:, 2 * j : 2 * j + 2, :])

            # ---- expb streaming (gpsimd queue): tiles of 4 s_k chunks ----
            expb_tiles = [None] * 8

            def emit_expb(t):
                tl = expb_pool.tile([128, 4, SUPLEN], BF16, name="expb")
                sup, g = divmod(t, 4)
                src = expb_d[:, sup * SUPLEN : (sup + 1) * SUPLEN].rearrange(
                    "(c p) q -> p c q", p=128
                )
                for j in range(4):
                    for hf in range(2):
                        nc.gpsimd.dma_start(
                            tl[:, j, hf * NB : (hf + 1) * NB],
                            src[:, g * 4 + j, hf * NB : (hf + 1) * NB],
                        )
                expb_tiles[t] = tl

            emit_expb(0)
            emit_expb(1)
            xv0 = x_half(xv_d, 0, nc.gpsimd)
            emit_expb(2)
            xk1 = x_half(xk_d, 1, nc.sync)
            emit_expb(3)
            xv1 = x_half(xv_d, 1, nc.gpsimd)
            xq1 = x_half(xq_d, 1, nc.gpsimd, split=1)
            nc.gpsimd.dma_start(wo_sb, wo_d[:, :, :])

            # ---- projection units: one (nt, mt) pair = 8 matmuls + evac ----
            def qk_proj(xt, half, w_sb, b_sb, dst, nt, mt):
                # nt is the global token block; xt holds columns of `half`
                lsl = slice(nt * NB - half * SUPLEN, (nt + 1) * NB - half * SUPLEN)
                csl = slice(nt * NB, (nt + 1) * NB)
                ps = pop.tile([128, NB], F32, name="ps_p", tag="pop")
                for kc in range(KC):
                    nc.tensor.matmul(
                        ps,
                        lhsT=w_sb[:, kc, mt * 128 : (mt + 1) * 128],
                        rhs=xt[:, kc, lsl],
                        start=(kc == 0),
                        stop=(kc == KC - 1),
                    )
                if dst is qT:
                    nc.vector.tensor_scalar_add(
                        qT[mt][:, csl], ps, scalar1=b_sb[:, mt : mt + 1]
                    )
                else:  # split into zero-padded per-head kT tiles
                    h0, h1 = 2 * mt, 2 * mt + 1
                    nc.vector.tensor_scalar_add(
                        kTh[h0][0:64, csl], ps[0:64, :],
                        scalar1=b_sb[0:64, mt : mt + 1],
                    )
                    nc.vector.tensor_scalar_add(
                        kTh[h1][64:128, csl], ps[64:128, :],
                        scalar1=b_sb[64:128, mt : mt + 1],
                    )

            def v_proj(xt, tk):
                # xt holds the half containing s_k chunk tk
                lsl = slice((tk % 8) * 128, (tk % 8 + 1) * 128)
                ps = pop.tile([128, CD], F32, name="ps_v", tag="pop")
                for kc in range(KC):
                    nc.tensor.matmul(
                        ps,
                        lhsT=xt[:, kc, lsl],
                        rhs=wv_sb[:, kc, :],
                        start=(kc == 0),
                        stop=(kc == KC - 1),
                    )
                psh = ps.rearrange("p (h d) -> p h d", h=HPC)
                # Wv host-reordered to vh order (0,2,1,3): first two blocks are
                # the even-parity heads (cols 0:64), last two odd (cols 64:128)
                nc.vector.tensor_copy(vv[:, 0:2, tk, 0:DH], psh[:, 0:2, :])
                nc.vector.tensor_copy(vv[:, 2:4, tk, DH:128], psh[:, 2:4, :])

            # minimal prefix: exactly what h0's first score chunks consume
            # (qT[0] sup0 columns and kTh[0]/[1] first quarter)
            qk_proj(xq0, 0, wq_sb, bq_sb, qT, 0, 0)
            qk_proj(xq0, 0, wq_sb, bq_sb, qT, 1, 0)
            qk_proj(xk0, 0, wk_sb, bk_sb, kTh, 0, 0)
            # remaining big memsets ride the DVE queue behind the prefix evacs
            nc.vector.memset(vv, 0.0)
            nc.vector.memset(vv[:, 0:2, :, 64:65], 1.0)
            nc.vector.memset(vv[:, 2:4, :, 0:1], 1.0)
            nc.vector.memset(kTh[2], 0.0)
            nc.vector.memset(kTh[3], 0.0)

            # everything else is deferred, injected into the attention stream
            # as its DMAs land (the input load is DMA-bandwidth-bound early on)
            inject = {
                (0, 0): {
                    0: [("k", xk0, 0, 1, 0)],
                    4: [("v", xv0, 0)],
                    5: [("k", xk1, 1, 2, 0), ("v", xv0, 1)],
                    6: [("v", xv0, 2)],
                    7: [("k", xk1, 1, 3, 0), ("v", xv0, 3)],
                    8: [("v", xv0, 4)],
                    9: [("v", xv0, 5)],
                    10: [("v", xv0, 6), ("v", xv0, 7)],
                    11: [("v", xv1, 8), ("v", xv1, 9)],
                    12: [("v", xv1, 10), ("v", xv1, 11)],
                    13: [("v", xv1, 12), ("v", xv1, 13)],
                    14: [("v", xv1, 14), ("v", xv1, 15)],
                },
                (0, 1): {
                    1: [("q", xq0, 0, 0, 1)],
                    3: [("q", xq0, 0, 1, 1)],
                    5: [("k", xk0, 0, 0, 1)],
                    7: [("k", xk0, 0, 1, 1)],
                    9: [("k", xk1, 1, 2, 1)],
                    11: [("k", xk1, 1, 3, 1)],
                },
                (0, 2): {
                    1: [("q", xq1, 1, 2, 0)],
                    3: [("q", xq1, 1, 2, 1)],
                    5: [("q", xq1, 1, 3, 0)],
                    7: [("q", xq1, 1, 3, 1)],
                },
            }

            def emit_injected(units):
                for u in units:
                    if u[0] == "v":
                        v_proj(u[1], u[2])
                    elif u[0] == "k":
                        qk_proj(u[1], u[2], wk_sb, bk_sb, kTh, u[3], u[4])
                    else:
                        qk_proj(u[1], u[2], wq_sb, bq_sb, qT, u[3], u[4])

            # ---- outproj ----
            outproj_queue = [(mo, nt) for nt in range(2) for mo in range(D // 128)]
            outproj_queue += [(mo, nt) for nt in range(2, 4) for mo in range(D // 128)]
            op_cursor = [0]
            OP_SUP0_TILES = 16

            def emit_outproj(limit, n=1, tail=False):
                for i in range(n):
                    if op_cursor[0] >= limit:
                        return
                    mo, nt = outproj_queue[op_cursor[0]]
                    op_cursor[0] += 1
                    csl = slice(nt * NB, (nt + 1) * NB)
                    if tail and i % 2 == 1:
                        # attention PSUM pools are free in the tail: 4-deep
                        psb = psc.tile([128, SUPLEN], F32, name="sc", tag="psc")
                        ps = psb[:, 0:NB]
                    else:
                        ps = pop.tile([128, NB], F32, name="ps_o", tag="pop")
                    for kc in range(2):
                        nc.tensor.matmul(
                            ps,
                            lhsT=wo_sb[:, kc, mo * 128 : (mo + 1) * 128],
                            rhs=cc[kc][:, csl],
                            start=(kc == 0),
                            stop=(kc == 1),
                        )
                    ot = oev_pool.tile([128, NB], BF16, name="ot")
                    if tail and i % 2 == 1:
                        nc.scalar.copy(ot, ps)  # ScalarE is idle in the tail
                    else:
                        nc.vector.tensor_copy(ot, ps)
                    nc.sync.dma_start(pout_d[mo * 128 : (mo + 1) * 128, csl], ot)

            # ---- epilogue (3 stages, pipelined into the next head) ----
            # the v-bias is folded into bo on the host (bv @ Wo.T), so the
            # normalize is a single bf16 multiply per head
            def make_epilogue(sup, h, o2s):
                mt, hh = h // 2, h % 2
                prow = slice(hh * 64, (hh + 1) * 64)
                den = 64 if hh == 0 else 0
                qsl = slice(sup * SUPLEN, (sup + 1) * SUPLEN)
                st = {}

                def s1():
                    # den row -> DRAM -> [128, 8] spread (partition-crossing
                    # APs require a DRAM bounce)
                    rsd = dram_pool.tile([1, SUPLEN], BF16, name="rsd")
                    nc.sync.dma_start(rsd, o2s[den : den + 1, :])
                    spread = spr_pool.tile([128, SUPLEN // 128], BF16, name="spread")
                    nc.sync.dma_start(
                        spread, rsd[:, :].rearrange("a (p f) -> (a p) f", p=128)
                    )
                    st["spread"] = spread

                def s2():
                    with nc.allow_low_precision(reason="softmax denom tolerates bf16"):
                        nc.vector.reciprocal(st["spread"], st["spread"])
                    rsd2 = dram_pool.tile([1, SUPLEN], BF16, name="rsd2")
                    nc.sync.dma_start(
                        rsd2[:, :].rearrange("a (p f) -> (a p) f", p=128), st["spread"]
                    )
                    rbt = rb_pool.tile([128, SUPLEN], BF16, name="rbt")
                    nc.sync.dma_start(rbt[prow, :], rsd2[:, :].partition_broadcast(64))
                    st["rbt"] = rbt

                def s3():
                    seg = cc[mt][prow, qsl]
                    nc.vector.tensor_mul(seg, o2s[prow, :], st["rbt"][prow, :])

                return [s1, s2, s3]

            # ---- attention ----
            pending = None
            for sup in range(SUPS):
                for h in range(HPC):
                    mt, hh = h // 2, h % 2
                    vh = {0: 0, 2: 1, 1: 2, 3: 3}[h]
                    out2 = pacc.tile([128, SUPLEN], F32, name="out2", tag="pacc")
                    a_tiles = [None] * TC
                    lag = 6 if (sup == 0 and h == 0) else 2

                    def attnv(ck):
                        for hf in range(2):
                            hsl = slice(hf * NB, (hf + 1) * NB)
                            nc.tensor.matmul(
                                out2[:, hsl],
                                lhsT=vv[:, vh, ck, :],
                                rhs=a_tiles[ck][:, hsl],
                                start=(ck == 0),
                                stop=(ck == TC - 1),
                            )
                        a_tiles[ck] = None

                    for ck in range(TC):
                        if sup == 0 and h == 3 and ck in (0, 4, 8, 12):
                            emit_expb(4 + ck // 4)  # prefetch sup1 expb
                        t = sup * 4 + ck // 4
                        sc = psc.tile([128, SUPLEN], F32, name="sc", tag="psc")
                        lhsT_k = kTh[h][:, ck * 128 : (ck + 1) * 128]
                        for hf in range(2):
                            hsl = slice(hf * NB, (hf + 1) * NB)
                            nc.tensor.matmul(
                                sc[:, hsl],
                                lhsT=lhsT_k,
                                rhs=qT[mt][:, sup * SUPLEN + hf * NB : sup * SUPLEN + (hf + 1) * NB],
                                start=True,
                                stop=True,
                            )
                        e = e_pool.tile([128, SUPLEN], BF16, name="e")
                        nc.scalar.activation(
                            e, sc, func=mybir.ActivationFunctionType.Exp,
                            bias=bias_t[:, 0:1], scale=EXP_SCALE,
                        )
                        a = a_pool.tile([128, SUPLEN], BF16, name="a")
                        nc.vector.tensor_mul(a, e, expb_tiles[t][:, ck % 4, :])
                        a_tiles[ck] = a
                        # deferred projection units ride the early streams
                        # (before this chunk's lagged attnv, which may consume
                        # their outputs)
                        units = inject.get((sup, h), {}).get(ck)
                        if units:
                            emit_injected(units)
                        if ck >= lag:
                            attnv(ck - lag)
                        # previous head's epilogue
                        if pending is not None:
                            if ck == 3:
                                pending[0]()
                            elif ck == 6:
                                pending[1]()
                            elif ck == 9:
                                pending[2]()
                                pending = None
                        # outproj interleave during sup1
                        if sup == 1 and ck in (2, 4, 6, 8, 10, 12) and (h > 0 or ck > 9):
                            emit_outproj(OP_SUP0_TILES, 1)
                    for ck in range(TC - lag, TC):
                        attnv(ck)
                    # evacuate out2 so the PSUM slot frees for the next head
                    o2s = o2s_pool.tile([128, SUPLEN], BF16, name="o2s")
                    nc.vector.tensor_copy(o2s, out2)
                    if pending is not None:
                        for f in pending:
                            f()
                    pending = make_epilogue(sup, h, o2s)

            # tail: final epilogue + remaining outproj tiles
            for f in pending:
                f()
            pending = None
            emit_outproj(len(outproj_queue), len(outproj_queue), tail=True)

            if debug:
                for m in range(2):
                    nc.sync.dma_start(dbg_qT[m, :, :], qT[m])
                    nc.sync.dma_start(dbg_cc[m, :, :], cc[m])
                for h in range(HPC):
                    nc.sync.dma_start(dbg_kT[h, :, :], kTh[h])
                nc.sync.dma_start(dbg_vv[:, :], vv.rearrange("p a b c -> p (a b c)"))

    nc.finalize()
    return nc


def make_in_maps(query, key, value, mask, chemical_bias, Wq, bq, Wk, bk, Wv, bv, Wo):
    """Host-side preprocessing: per-core input dicts (8 cores)."""
    f32 = np.float32

    def xarr(x):
        # [S, D] -> [128, KC, S]: arr[p, kc, s] = x[s, kc*128+p]
        return np.ascontiguousarray(
            np.asarray(x, f32).T.reshape(KC, 128, S).transpose(1, 0, 2)
        ).astype(nbf16)

    per_batch = []
    for b in range(B):
        xq = xarr(query[b])
        xk = xarr(key[b])
        xv = xarr(value[b])
        bm = np.where(mask[b, 0] == 0, f32(0.0), np.exp(chemical_bias[b], dtype=f32))
        expbT = np.ascontiguousarray(bm.T, dtype=nbf16)  # [s_k, s_q]
        per_batch.append((xq, xk, xv, expbT))

    def warr(wt):
        # [D, CD] -> [128, KC, CD]
        return np.ascontiguousarray(
            np.asarray(wt, f32).reshape(KC, 128, CD).transpose(1, 0, 2)
        ).astype(nbf16)

    per_group = []
    for g in range(4):
        hsl = slice(g * CD, (g + 1) * CD)
        wq_ = warr(Wq[hsl].T)
        wk_ = warr(Wk[hsl].T)
        # Wv columns reordered to vh head order (0,2,1,3) for contiguous evacs
        wv_full = np.asarray(Wv[hsl].T, f32).reshape(D, HPC, DH)
        wv_ = warr(np.ascontiguousarray(wv_full[:, [0, 2, 1, 3], :]).reshape(D, CD))
        wo_ = np.ascontiguousarray(
            np.asarray(Wo[:, hsl].T, f32).reshape(2, 128, D).transpose(1, 0, 2)
        ).astype(nbf16)
        bq_ = np.ascontiguousarray(np.asarray(bq[hsl], f32).reshape(2, 128).T)
        bk_ = np.ascontiguousarray(np.asarray(bk[hsl], f32).reshape(2, 128).T)
        per_group.append((wq_, wk_, wv_, wo_, bq_, bk_))

    in_maps = []
    for core in range(NCORES):
        b, g = divmod(core, 4)
        xq, xk, xv, expbT = per_batch[b]
        wq_, wk_, wv_, wo_, bq_, bk_ = per_group[g]
        in_maps.append(
            {
                "xq": xq, "xk": xk, "xv": xv,
                "wq": wq_, "wk": wk_, "wv": wv_, "wo": wo_,
                "bq": bq_, "bk": bk_,
                "expb": expbT,
            }
        )
    return in_maps


def combine_outputs(results, bo):
    """Sum per-group transposed bf16 partials into the full [B, S, D] output."""
    out = np.empty((B, S, D), np.float32)
    for b in range(B):
        acc = results[4 * b]["pout"].astype(np.float32)
        for g in range(1, 4):
            acc = acc + results[4 * b + g]["pout"].astype(np.float32)
        out[b] = acc.T + np.asarray(bo, np.float32)
    return out


_NC_CACHE = {}


def _get_module(debug=False):
    if debug not in _NC_CACHE:
        _NC_CACHE[debug] = build_module(debug=debug)
    return _NC_CACHE[debug]


def run_spmd(in_maps, debug=False, **kwargs):
    from concourse.bass_utils import run_bass_kernel_spmd

    nc = _get_module(debug)
    return run_bass_kernel_spmd(nc, in_maps, core_ids=list(range(NCORES)), **kwargs)


def kernel(query, key, value, mask, chemical_bias, Wq, bq, Wk, bk, Wv, bv, Wo, bo):
    in_maps = make_in_maps(
        query, key, value, mask, chemical_bias, Wq, bq, Wk, bk, Wv, bv, Wo
    )
    res = run_spmd(in_maps)
    # the per-head v-bias rides through attention unchanged (weights sum to
    # 1), so its contribution is the constant bv @ Wo.T folded into bo here
    bo_eff = np.asarray(bo, np.float32) + np.asarray(bv, np.float32) @ np.asarray(Wo, np.float32).T
    return combine_outputs(res.results, bo_eff)


# revision 38
# speedup vs baseline: 1.0550x; 1.0550x over previous
"""MultiHeadAttention TRN2 Bass kernel, sharded over 8 NeuronCores.

Sharding: 8 cores = 2 batches x 4 head-groups. Each core computes 4 heads of
one batch end-to-end (q/k/v projections, biased+masked softmax attention, and
a partial output projection); the host sums the per-group partial outputs.

On-device layout is fully "transposed" so no on-device transposes are needed:
  - host supplies x^T [D, S] per batch (bf16) and per-core weight slices
  - projections produce qT/kT [head_dims, S]; v stays natural [S, head_dims]
  - scores are computed transposed: scoresT[s_k, s_q] = kT.T @ qT per head,
    with the other head's kT rows zeroed so every matmul contracts K=128
  - softmax: exp on ScalarE (PSUM->SBUF), bias/mask applied as a multiply
    with host-precomputed exp(bias_masked)^T on VectorE, and the denominator
    comes free as an extra ones-column in the attn@v matmul
  - attn@v: out2[dh+1, s_q] accumulated over s_k chunks; normalization by the
    ones-row + per-head v-bias correction happens on the way into the concat
    tile; output projection emits partial_out^T [D, S] (f32) per core.
"""

import numpy as np
import ml_dtypes

import concourse.bass as bass
import concourse.mybir as mybir
import concourse.tile as tile
from concourse.bacc import Bacc

BF16 = mybir.dt.bfloat16
F32 = mybir.dt.float32
nbf16 = ml_dtypes.bfloat16

B = 2
S_FULL = 2048
D = 1024
H = 16
DH = 64
HPC = 4  # heads per core
CD = HPC * DH  # 256 per-core projected dims
NCORES = 8
SCALE = 8.0  # sqrt(DH)

KC = D // 128  # 8 contraction chunks for projections
NB = 512  # projection token-block (free dim per matmul)


def build_module(S=S_FULL, debug=False):
    """Build the single-core Bass program (same program runs SPMD on 8 cores)."""
    assert S % 1024 == 0
    SUPS = 2  # s_q superblocks
    SUPLEN = S // SUPS  # columns per superblock
    NHALF = SUPLEN // NB  # matmuls per psum row-tile
    NT = S // NB  # projection token blocks
    TC = S // 128  # token / s_k chunks

    # Bacc (not plain Bass): its compile() splits multi-wait instructions to
    # the 1-wait HW limit and inserts library/ACT-table loads, which the
    # neuronxcc walrus codegen path requires.
    nc = Bacc(None)

    xqT = nc.dram_tensor("xqT", [D, S], BF16, kind="ExternalInput")
    xkT = nc.dram_tensor("xkT", [D, S], BF16, kind="ExternalInput")
    xvT = nc.dram_tensor("xvT", [D, S], BF16, kind="ExternalInput")
    # weights arrive pre-arranged [128, kc*CD] so the load is one fully
    # contiguous-per-partition DMA
    wqT = nc.dram_tensor("wqT", [128, KC * CD], BF16, kind="ExternalInput")
    wkT = nc.dram_tensor("wkT", [128, KC * CD], BF16, kind="ExternalInput")
    wvT = nc.dram_tensor("wvT", [128, KC * CD], BF16, kind="ExternalInput")
    woT = nc.dram_tensor("woT", [128, (CD // 128) * D], BF16, kind="ExternalInput")
    bqc = nc.dram_tensor("bqc", [128, 2], F32, kind="ExternalInput")
    bkc = nc.dram_tensor("bkc", [128, 2], F32, kind="ExternalInput")
    bvc = nc.dram_tensor("bvc", [64, HPC], F32, kind="ExternalInput")
    expbT = nc.dram_tensor("expbT", [S, S], BF16, kind="ExternalInput")
    poutT = nc.dram_tensor("poutT", [D, S], BF16, kind="ExternalOutput")
    if debug:
        TCD = S // 128
        dbg_qT = nc.dram_tensor("dbg_qT", [2, 128, S], BF16, kind="ExternalOutput")
        dbg_kTh = nc.dram_tensor("dbg_kTh", [HPC, 128, S], BF16, kind="ExternalOutput")
        dbg_vv = nc.dram_tensor(
            "dbg_vv", [128, HPC * TCD * (DH + 1)], BF16, kind="ExternalOutput"
        )
        dbg_cc = nc.dram_tensor("dbg_cc", [2, 128, S], BF16, kind="ExternalOutput")
        dbg_rb = nc.dram_tensor("dbg_rb", [2, HPC, 64, S // 2], F32, kind="ExternalOutput")
        dbg_ea = nc.dram_tensor("dbg_ea", [2, 128, S // 2], BF16, kind="ExternalOutput")

    with tile.TileContext(nc) as tc:
        with (
            tc.tile_pool(name="statics", bufs=1) as statics,
            tc.tile_pool(name="xs", bufs=9) as xs_pool,
            tc.tile_pool(name="xv", bufs=KC) as xv_pool,
            tc.tile_pool(name="expb", bufs=2) as expb_pool,
            tc.tile_pool(name="e", bufs=3) as e_pool,
            tc.tile_pool(name="a", bufs=3) as a_pool,
            tc.tile_pool(name="rec", bufs=2) as rec_pool,
            tc.tile_pool(name="spr", bufs=2) as spread_pool,
            tc.tile_pool(name="rb", bufs=2) as rb_pool,
            tc.tile_pool(name="segt", bufs=2) as seg_pool,
            tc.tile_pool(name="oev", bufs=3) as oev_pool,
            tc.tile_pool(name="psc", bufs=2, space="PSUM") as psc,
            tc.tile_pool(name="pacc", bufs=2, space="PSUM") as pacc,
            tc.tile_pool(name="dsc", bufs=4, space="DRAM") as dram_pool,
        ):
            # ---- static tiles ----
            wq_sb = statics.tile([128, KC, CD], BF16, name="wq_sb")
            wk_sb = statics.tile([128, KC, CD], BF16, name="wk_sb")
            wv_sb = statics.tile([128, KC, CD], BF16, name="wv_sb")
            wo_sb = statics.tile([128, CD // 128, D], BF16, name="wo_sb")
            bq_sb = statics.tile([128, 2], F32, name="bq_sb")
            bk_sb = statics.tile([128, 2], F32, name="bk_sb")
            bv_sb = statics.tile([64, HPC], F32, name="bv_sb")
            qT = [statics.tile([128, S], BF16, name=f"qT{m}") for m in range(2)]
            # per-head kT, zero-padded on the other head's 64 rows so scores
            # matmuls contract a full K=128
            kTh = [statics.tile([128, S], BF16, name=f"kTh{h}") for h in range(HPC)]
            vv = statics.tile([128, HPC, TC, DH + 1], BF16, name="vv")
            cc = [statics.tile([128, S], BF16, name=f"cc{m}") for m in range(2)]

            nc.sync.dma_start(wq_sb, wqT[:, :].rearrange("p (kc m) -> p kc m", kc=KC))
            nc.sync.dma_start(wk_sb, wkT[:, :].rearrange("p (kc m) -> p kc m", kc=KC))
            nc.sync.dma_start(wv_sb, wvT[:, :].rearrange("p (kc m) -> p kc m", kc=KC))
            nc.sync.dma_start(
                wo_sb, woT[:, :].rearrange("p (kc m) -> p kc m", kc=CD // 128)
            )
            nc.sync.dma_start(bq_sb, bqc[:, :])
            nc.sync.dma_start(bk_sb, bkc[:, :])
            nc.sync.dma_start(bv_sb, bvc[:, :])

            # zero-fills on the otherwise-idle GpSimd engine
            for h in range(HPC):
                nc.gpsimd.memset(kTh[h], 0.0)
            nc.gpsimd.memset(vv[:, :, :, DH : DH + 1], 1.0)

            # ---- phase 1: q/k projections (transposed outputs) ----
            for xdram, w_sb, b_sb, is_q in (
                (xqT, wq_sb, bq_sb, True),
                (xkT, wk_sb, bk_sb, False),
            ):
                for half in range(NT // 2):
                    xts2 = []
                    for kc in range(KC):
                        xt = xs_pool.tile([128, 2 * NB], BF16, name="xt")
                        nc.sync.dma_start(
                            xt,
                            xdram[
                                kc * 128 : (kc + 1) * 128,
                                half * 2 * NB : (half + 1) * 2 * NB,
                            ],
                        )
                        xts2.append(xt)
                    for nt2 in range(2):
                        nt = half * 2 + nt2
                        xts = [t[:, nt2 * NB : (nt2 + 1) * NB] for t in xts2]
                        for mt in range(2):
                            ps = psc.tile([128, NB], F32, name="ps_proj", tag="psc")
                            for kc in range(KC):
                                nc.tensor.matmul(
                                    ps,
                                    lhsT=w_sb[:, kc, mt * 128 : (mt + 1) * 128],
                                    rhs=xts[kc],
                                    start=(kc == 0),
                                    stop=(kc == KC - 1),
                                )
                            # evacuate on DVE (tensor_scalar add, per-partition
                            # bias) to keep ScalarE free for the exp stream
                            csl = slice(nt * NB, (nt + 1) * NB)
                            if is_q:
                                nc.vector.tensor_scalar_add(
                                    qT[mt][:, csl], ps, scalar1=b_sb[:, mt : mt + 1]
                                )
                            else:
                                # split the head-pair psum into the zero-padded
                                # per-head kT tiles (lane-aligned halves)
                                h0, h1 = 2 * mt, 2 * mt + 1
                                nc.vector.tensor_scalar_add(
                                    kTh[h0][0:64, csl],
                                    ps[0:64, :],
                                    scalar1=b_sb[0:64, mt : mt + 1],
                                )
                                nc.vector.tensor_scalar_add(
                                    kTh[h1][64:128, csl],
                                    ps[64:128, :],
                                    scalar1=b_sb[64:128, mt : mt + 1],
                                )

            # ---- phase 1b: v projection (natural layout, no bias) ----
            xv_tiles = []
            for kc in range(KC):
                xt = xv_pool.tile([128, S], BF16, name="xvt")
                nc.sync.dma_start(xt, xvT[kc * 128 : (kc + 1) * 128, :])
                xv_tiles.append(xt)

            # exp(bias_masked)^T superblocks — emitted after ALL projection
            # x loads so those win the DMA queues at kernel start. One DMA
            # per s_k chunk: Tile's subtile deps then let attention chunk ck
            # start as soon as ITS slice has landed.
            expb_tiles = []
            for sup in range(SUPS):
                t = expb_pool.tile([128, TC, SUPLEN], BF16, name="expb")
                src = expbT[:, sup * SUPLEN : (sup + 1) * SUPLEN].rearrange(
                    "(c p) q -> p c q", p=128
                )
                for ckd in range(TC):
                    nc.sync.dma_start(t[:, ckd, :], src[:, ckd, :])
                expb_tiles.append(t)
            for tk in range(TC):
                ps = pacc.tile([128, CD], F32, name="ps_v", tag="pacc")
                for kc in range(KC):
                    nc.tensor.matmul(
                        ps,
                        lhsT=xv_tiles[kc][:, tk * 128 : (tk + 1) * 128],
                        rhs=wv_sb[:, kc, :],
                        start=(kc == 0),
                        stop=(kc == KC - 1),
                    )
                nc.vector.tensor_copy(
                    vv[:, :, tk, 0:DH],
                    ps.rearrange("p (h d) -> p h d", h=HPC),
                )

            # ---- output projection tile emitter (used from phase 2 + tail) ----
            op_serial = [0]

            def outproj_tile(mo, nt, evac_engine=None):
                # alternate between the two PSUM pools for 4-deep pipelining
                i = op_serial[0]
                op_serial[0] += 1
                pool, tag = (psc, "psc") if i % 2 == 0 else (pacc, "pacc")
                if evac_engine == "vector":
                    pool, tag = pacc, "pacc"  # attention keeps psc for scores
                ps = pool.tile([128, NB], F32, name="ps_o", tag=tag)
                for kc in range(CD // 128):
                    nc.tensor.matmul(
                        ps,
                        lhsT=wo_sb[:, kc, mo * 128 : (mo + 1) * 128],
                        rhs=cc[kc][:, nt * NB : (nt + 1) * NB],
                        start=(kc == 0),
                        stop=(kc == CD // 128 - 1),
                    )
                ot = oev_pool.tile([128, NB], BF16, name="ot")
                if evac_engine == "vector" or (evac_engine is None and i % 2 == 0):
                    nc.vector.tensor_copy(ot, ps)
                else:  # "scalar" or alternating default
                    nc.scalar.copy(ot, ps)
                nc.sync.dma_start(
                    poutT[mo * 128 : (mo + 1) * 128, nt * NB : (nt + 1) * NB], ot
                )

            # sup-0 outproj tiles get interleaved into sup-1's attention
            # (their cc columns are final once sup-0's last epilogue lands)
            op_queue = [(mo, nt) for nt in range(NT // 2) for mo in range(D // 128)]

            # ---- phase 2: attention ----
            # The epilogue (normalize-by-sum) of instance i-1 is software-
            # pipelined into instance i's chunk loop in three stages so the
            # reciprocal/broadcast DMA chain never stalls the in-order DVE
            # stream that feeds PE with A tiles.
            def make_epilogue(sup, h, out2):
                qsl = slice(sup * SUPLEN, (sup + 1) * SUPLEN)
                mt = h // 2
                st = {}

                def s1():
                    # sum row PSUM->SBUF, then spread the 1xN row across 128
                    # partitions via DRAM so the reciprocal runs wide
                    st["ssum"] = rec_pool.tile([DH + 1, SUPLEN], F32, name="ssum")
                    nc.vector.tensor_copy(
                        st["ssum"][DH : DH + 1, :], out2[DH : DH + 1, :]
                    )
                    st["rsd"] = dram_pool.tile([1, SUPLEN], F32, name="rsd")
                    nc.sync.dma_start(st["rsd"], st["ssum"][DH : DH + 1, :])
                    st["spread"] = spread_pool.tile([128, SUPLEN // 128], F32, name="spread")
                    nc.sync.dma_start(
                        st["spread"],
                        st["rsd"][:, :].rearrange("a (p f) -> (a p) f", p=128),
                    )

                def s2():
                    nc.vector.reciprocal(st["spread"], st["spread"])
                    st["rsd2"] = dram_pool.tile([1, SUPLEN], F32, name="rsd2")
                    nc.sync.dma_start(
                        st["rsd2"][:, :].rearrange("a (p f) -> (a p) f", p=128),
                        st["spread"],
                    )
                    st["rb"] = rb_pool.tile([64, SUPLEN], F32, name="rb")
                    nc.sync.dma_start(
                        st["rb"], st["rsd2"][:, :].partition_broadcast(64)
                    )

                def s3():
                    rb = st["rb"]
                    if debug:
                        nc.sync.dma_start(dbg_rb[sup, h, :, :], rb)
                    if h % 2 == 0:
                        seg = cc[mt][0:64, qsl]
                        nc.vector.tensor_mul(seg, out2[0:DH, :], rb)
                        nc.vector.tensor_scalar_add(
                            seg, seg, scalar1=bv_sb[:, h : h + 1]
                        )
                    else:
                        segt = seg_pool.tile([64, SUPLEN], BF16, name="segt")
                        nc.vector.tensor_mul(segt, out2[0:DH, :], rb)
                        nc.vector.tensor_scalar_add(
                            segt, segt, scalar1=bv_sb[:, h : h + 1]
                        )
                        # partition move 0-63 -> 64-127 via DMA
                        nc.sync.dma_start(cc[mt][64:128, qsl], segt)

                return (s1, s2, s3)

            pending = None
            for sup in range(SUPS):
                for h in range(HPC):
                    mt = h // 2
                    out2 = pacc.tile([DH + 1, SUPLEN], F32, name="out2", tag="pacc")
                    for ck in range(TC):
                        sc = psc.tile([128, SUPLEN], F32, name="sc", tag="psc")
                        lhsT_k = kTh[h][:, ck * 128 : (ck + 1) * 128]
                        for hf in range(NHALF):
                            hsl = slice(hf * NB, (hf + 1) * NB)
                            nc.tensor.matmul(
                                sc[:, hsl],
                                lhsT=lhsT_k,
                                rhs=qT[mt][:, sup * SUPLEN + hf * NB : sup * SUPLEN + (hf + 1) * NB],
                                start=True,
                                stop=True,
                            )
                        e = e_pool.tile([128, SUPLEN], BF16, name="e")
                        nc.scalar.activation(
                            e, sc, func=mybir.ActivationFunctionType.Exp
                        )
                        a = a_pool.tile([128, SUPLEN], BF16, name="a")
                        nc.vector.tensor_mul(a, e, expb_tiles[sup][:, ck, :])
                        if debug and sup == 0 and h == 0 and ck == 0:
                            nc.sync.dma_start(dbg_ea[0, :, :], e)
                            nc.sync.dma_start(dbg_ea[1, :, :], a)
                        for hf in range(NHALF):
                            hsl = slice(hf * NB, (hf + 1) * NB)
                            nc.tensor.matmul(
                                out2[:, hsl],
                                lhsT=vv[:, h, ck, :],
                                rhs=a[:, hsl],
                                start=(ck == 0),
                                stop=(ck == TC - 1),
                            )
                        if pending is not None:
                            if ck == 0:
                                pending[0]()
                            elif ck == TC // 4:
                                pending[1]()
                            elif ck == TC // 2:
                                pending[2]()
                        # (outproj interleave into attention measured net-negative:
                        # its DVE evacs stall the A-mul stream that feeds PE)
                    pending = make_epilogue(sup, h, out2)
            if debug:
                for stage in pending:
                    stage()
                pending = None
                for m in range(2):
                    nc.sync.dma_start(dbg_qT[m, :, :], qT[m])
                    nc.sync.dma_start(dbg_cc[m, :, :], cc[m])
                for h in range(HPC):
                    nc.sync.dma_start(dbg_kTh[h, :, :], kTh[h])
                nc.sync.dma_start(dbg_vv[:, :], vv.rearrange("p a b c -> p (a b c)"))

            # ---- phase 3: output projection tail ----
            # remaining sup-0 tiles (if any) + all sup-1 tiles, with the final
            # attention instance's epilogue stages interleaved into the first
            # few so PE never idles waiting for the last normalize
            tail_tiles = op_queue + [
        (mo, nt) for nt in range(NT // 2, NT) for mo in range(D // 128)
            ]
            emitted = 0
            stages_done = 0
            for mo, nt in tail_tiles:
                outproj_tile(mo, nt, evac_engine="scalar" if emitted < 10 else None)
                emitted += 1
                if pending is not None and stages_done < 3 and emitted % 2 == 0:
                    pending[stages_done]()
                    stages_done += 1
            if pending is not None:
                while stages_done < 3:
                    pending[stages_done]()
                    stages_done += 1
                pending = None

    nc.finalize()  # runs Bacc.compile(): wait-splitting, reg alloc, table loads
    return nc


def make_in_maps(query, key, value, mask, chemical_bias, Wq, bq, Wk, bk, Wv, bv, Wo, S=S_FULL):
    """Host-side preprocessing: per-core input dicts (8 cores)."""
    f32 = np.float32

    def c(a, dt):
        return np.ascontiguousarray(a, dtype=dt)

    per_batch = []
    for b in range(B):
        xq = c(query[b].T, nbf16)
        xk = c(key[b].T, nbf16)
        xv = c(value[b].T, nbf16)
        bm = np.where(mask[b, 0] == 0, f32(0.0), np.exp(chemical_bias[b], dtype=f32))
        expbT_ = c(bm.T, nbf16)
        per_batch.append((xq, xk, xv, expbT_))

    def warr(wt, kc):
        # [kc*128, M] -> [128, kc*M]: per-partition-contiguous device layout
        m = wt.shape[1]
        return np.ascontiguousarray(
            wt.reshape(kc, 128, m).transpose(1, 0, 2).reshape(128, kc * m), nbf16
        )

    per_group = []
    for g in range(4):
        hsl = slice(g * CD, (g + 1) * CD)
        wqT_ = warr(np.asarray((Wq[hsl] / SCALE).T, np.float32), KC)
        wkT_ = warr(np.asarray(Wk[hsl].T, np.float32), KC)
        wvT_ = warr(np.asarray(Wv[hsl].T, np.float32), KC)
        woT_ = warr(np.asarray(Wo[:, hsl].T, np.float32), CD // 128)
        bqc_ = c((bq[hsl] / SCALE).reshape(2, 128).T, f32)
        bkc_ = c(bk[hsl].reshape(2, 128).T, f32)
        bvc_ = c(bv[hsl].reshape(HPC, 64).T, f32)
        per_group.append((wqT_, wkT_, wvT_, woT_, bqc_, bkc_, bvc_))

    in_maps = []
    for core in range(NCORES):
        b, g = divmod(core, 4)
        xq, xk, xv, expbT_ = per_batch[b]
        wqT_, wkT_, wvT_, woT_, bqc_, bkc_, bvc_ = per_group[g]
        in_maps.append(
            {
                "xqT": xq,
                "xkT": xk,
                "xvT": xv,
                "wqT": wqT_,
                "wkT": wkT_,
                "wvT": wvT_,
                "woT": woT_,
                "bqc": bqc_,
                "bkc": bkc_,
                "bvc": bvc_,
                "expbT": expbT_,
            }
        )
    return in_maps


def combine_outputs(results, bo):
    """Sum per-group transposed partials into the full [B, S, D] output."""
    out = np.empty((B, S_FULL, D), np.float32)
    for b in range(B):
        acc = results[4 * b]["poutT"].T.astype(np.float32).copy()
        for g in range(1, 4):
            acc += results[4 * b + g]["poutT"].T.astype(np.float32)
        out[b] = acc + bo.astype(np.float32)
    return out


_NC_CACHE = {}


def _get_module(S=S_FULL, debug=False):
    key = (S, debug)
    if key not in _NC_CACHE:
        _NC_CACHE[key] = build_module(S, debug=debug)
    return _NC_CACHE[key]


def run_spmd(in_maps, S=S_FULL, debug=False, **kwargs):
    from concourse.bass_utils import run_bass_kernel_spmd

    nc = _get_module(S, debug)
    return run_bass_kernel_spmd(nc, in_maps, core_ids=list(range(NCORES)), **kwargs)


def kernel(query, key, value, mask, chemical_bias, Wq, bq, Wk, bk, Wv, bv, Wo, bo):
    in_maps = make_in_maps(
        query, key, value, mask, chemical_bias, Wq, bq, Wk, bk, Wv, bv, Wo
    )
    res = run_spmd(in_maps)
    return combine_outputs(res.results, bo)

